# revision 3
# baseline (speedup 1.0000x reference)
"""Trainium2 Bass kernel for pairwise Mahalanobis adjacency.

Computes adj[b,i,j] = exp(-(x_i - x_j)^T (W W^T) (x_i - x_j)) + I
for regional_means x of shape (B=2, N=1024, C=64), W of shape (64, 64).

Algebra: with Z = X @ W and G = Z @ Z^T, d = diag(G):
    q[i,j] = d[i] + d[j] - 2 G[i,j]
    adj    = exp(2G - d_i - d_j) + I

Sharding (8 cores): core k handles batch b = k // 4, row slab
s = k % 4 -> rows [s*256, (s+1)*256).  Each core receives the full
X^T for its batch with columns rotated left by row0 = s*256 so that
the diagonal block sits at a fixed local position (identical SPMD
program on all cores); the host un-rotates when gathering.
"""

import numpy as np

import concourse.bass as bass
import concourse.tile as tile
from concourse import bacc, masks, mybir
from concourse.bass_utils import run_bass_kernel_spmd

B, N, C = 2, 1024, 64
SLAB = N // 4  # 256 rows per core
P = 128        # row-group size (SBUF/PSUM partitions)
NT = 512       # psum tile free size (max fp32 moving operand)
F32 = mybir.dt.float32

_NC = None
LAST_EXEC_NS = None
TRACE = False


def _ensure_ntff_hook():
    """Install the antenv.axon_hooks NTFF-profile shim if the image lacks it."""
    import sys
    import types

    try:
        from antenv.axon_hooks import get_axon_ntff_profile_hook  # noqa: F401

        return
    except ImportError:
        pass
    try:
        from trn_agent_boot.trn_boot import _ntff_profile_via_ctypes
    except ImportError:
        return
    hook = _ntff_profile_via_ctypes("/opt/axon/libaxon_pjrt.so")
    mod = types.ModuleType("antenv.axon_hooks")
    state = {"hook": hook}
    mod.get_axon_ntff_profile_hook = lambda: state["hook"]
    mod.set_axon_ntff_profile_hook = lambda h: state.__setitem__("hook", h)
    import antenv

    sys.modules["antenv.axon_hooks"] = mod
    antenv.axon_hooks = mod


def _build():
    nc = bacc.Bacc("TRN2", target_bir_lowering=False, debug=False, num_devices=8)
    xt_d = nc.dram_tensor("xt", [C, N], F32, kind="ExternalInput").ap()
    w_d = nc.dram_tensor("w", [C, C], F32, kind="ExternalInput").ap()
    out_d = nc.dram_tensor("out", [SLAB, N], F32, kind="ExternalOutput").ap()

    with tile.TileContext(nc) as tc:
        with (
            tc.tile_pool(name="singles", bufs=1) as singles,
            tc.tile_pool(name="psum", bufs=2, space="PSUM") as psum,
            tc.tile_pool(name="psmall", bufs=2, space="PSUM") as psmall,
            tc.tile_pool(name="outs", bufs=4) as outs,
        ):
            xt_sb = singles.tile([C, N], F32)
            nc.sync.dma_start(xt_sb[:], xt_d[:, :])
            w_sb = singles.tile([C, C], F32)
            nc.sync.dma_start(w_sb[:], w_d[:, :])

            eye = singles.tile([P, P], F32)
            masks.make_identity(nc, eye[:])

            ones_c = singles.tile([C, 1], F32)
            nc.vector.memset(ones_c[:], 1.0)
            ones_r = singles.tile([1, P], F32)
            nc.vector.memset(ones_r[:], 1.0)
            one_1 = singles.tile([1, 1], F32)
            nc.vector.memset(one_1[:], 1.0)

            # ZT[c', i] = sum_c W[c, c'] XT[c, i]  -> Z^T, [C, N]
            zt = singles.tile([C, N], F32)
            for jc in range(N // NT):
                pz = psum.tile([C, NT], F32, tag="pz")
                nc.tensor.matmul(
                    pz[:], w_sb[:], xt_sb[:, bass.ts(jc, NT)], start=True, stop=True
                )
                nc.vector.tensor_copy(zt[:, bass.ts(jc, NT)], pz[:])

            # sq = ZT * ZT ; nd[0, j] = -d[j] = -sum_c ZT[c, j]^2
            sq = singles.tile([C, N], F32)
            nc.vector.tensor_mul(sq[:], zt[:], zt[:])
            nd = singles.tile([1, N], F32)
            for jc in range(N // NT):
                pd = psmall.tile([1, NT], F32, tag="pd")
                nc.tensor.matmul(
                    pd[:], ones_c[:], sq[:, bass.ts(jc, NT)], start=True, stop=True
                )
                nc.scalar.mul(nd[:, bass.ts(jc, NT)], pd[:], -1.0)

            # ndi[:, g] = nd[0, g*128:(g+1)*128] transposed to partitions
            ndi = singles.tile([P, 2], F32)
            for g in range(2):
                pt = psmall.tile([P, 1], F32, tag="pt")
                nc.tensor.matmul(
                    pt[:], nd[0:1, bass.ts(g, P)], one_1[:], start=True, stop=True
                )
                nc.vector.tensor_copy(ndi[:, g : g + 1], pt[:])

            # zt2 = 2 * ZT[:, slab rows]
            zt2 = singles.tile([C, SLAB], F32)
            nc.scalar.mul(zt2[:], zt[:, 0:SLAB], 2.0)

            for g in range(2):
                for jc in range(N // NT):
                    pq = psum.tile([P, NT], F32, tag="pq")
                    # pq = 2 G[rows g, cols jc]
                    nc.tensor.matmul(
                        pq[:],
                        zt2[:, bass.ts(g, P)],
                        zt[:, bass.ts(jc, NT)],
                        start=True,
                        stop=False,
                    )
                    # pq += -d_j (rank-1 broadcast along rows)
                    nc.tensor.matmul(
                        pq[:],
                        ones_r[:],
                        nd[0:1, bass.ts(jc, NT)],
                        start=False,
                        stop=True,
                    )
                    ot = outs.tile([P, NT], F32, tag="ot")
                    # ot = exp(pq - d_i)
                    nc.scalar.activation(
                        ot[:],
                        pq[:],
                        mybir.ActivationFunctionType.Exp,
                        bias=ndi[:, g : g + 1],
                        scale=1.0,
                    )
                    if jc == 0:
                        # rotated diagonal block: local col == local row
                        nc.vector.tensor_add(
                            ot[:, bass.ts(g, P)], ot[:, bass.ts(g, P)], eye[:]
                        )
                    nc.sync.dma_start(out_d[bass.ts(g, P), bass.ts(jc, NT)], ot[:])

    nc.compile()
    return nc


def _get_nc():
    global _NC
    if _NC is None:
        _NC = _build()
    return _NC


def kernel(regional_means, W, c=None, **_kw):
    global LAST_EXEC_NS
    x = np.ascontiguousarray(np.asarray(regional_means, dtype=np.float32))
    w = np.ascontiguousarray(np.asarray(W, dtype=np.float32))
    assert x.shape == (B, N, C) and w.shape == (C, C)

    nc = _get_nc()
    in_maps = []
    for k in range(8):
        b, s = divmod(k, 4)
        row0 = s * SLAB
        xt_rot = np.roll(x[b].T, -row0, axis=1)
        in_maps.append({"xt": np.ascontiguousarray(xt_rot), "w": w})

    if TRACE:
        _ensure_ntff_hook()
    res = run_bass_kernel_spmd(nc, in_maps, core_ids=list(range(8)), trace=TRACE)
    LAST_EXEC_NS = res.exec_time_ns

    adj = np.empty((B, N, N), dtype=np.float32)
    for k in range(8):
        b, s = divmod(k, 4)
        row0 = s * SLAB
        adj[b, row0 : row0 + SLAB, :] = np.roll(res.results[k]["out"], row0, axis=1)
    return adj


# revision 6
# speedup vs baseline: 1.9778x; 1.9778x over previous
"""Trainium2 Bass kernel for pairwise Mahalanobis adjacency.

Computes adj[b,i,j] = exp(-(x_i - x_j)^T (W W^T) (x_i - x_j)) + I
for regional_means x of shape (B=2, N=1024, C=64), W of shape (64, 64).

Algebra: with Z = X @ W and G = Z @ Z^T, d = diag(G):
    q[i,j] = d[i] + d[j] - 2 G[i,j]
    adj    = exp(2G - d_i - d_j) + I

Sharding (8 cores): core k handles batch b = k // 4, row slab
s = k % 4 -> rows [s*256, (s+1)*256).  Each core receives the full
X^T for its batch with columns rotated left by row0 = s*256 so that
the diagonal block sits at a fixed local position (identical SPMD
program on all cores); the host un-rotates when gathering.

Compute is bf16 on the TensorEngine (single-pass matmuls).  The -d_j
row term rides in the same matmul via an augmented contraction row
(K=65: rows 0..63 = 2*Z^T, row 64 = ones x (-d)).  The -d_i column
term is the per-partition activation bias.  The diagonal is exact:
q_ii = 0 and exp(0)+1 = 2 exactly, enforced by an affine_select fill.
"""

import numpy as np
import ml_dtypes

import concourse.bass as bass
import concourse.tile as tile
from concourse import bacc, mybir
from concourse.bass_utils import run_bass_kernel_spmd

B, N, C = 2, 1024, 64
SLAB = N // 4  # 256 rows per core
P = 128        # row-group size (SBUF/PSUM partitions)
NT = 512       # psum tile free size
F32 = mybir.dt.float32
BF16 = mybir.dt.bfloat16

_NC = None
LAST_EXEC_NS = None
TRACE = False


def _ensure_ntff_hook():
    """Install the antenv.axon_hooks NTFF-profile shim if the image lacks it."""
    import sys
    import types

    try:
        from antenv.axon_hooks import get_axon_ntff_profile_hook  # noqa: F401

        return
    except ImportError:
        pass
    try:
        from trn_agent_boot.trn_boot import _ntff_profile_via_ctypes
    except ImportError:
        return
    hook = _ntff_profile_via_ctypes("/opt/axon/libaxon_pjrt.so")
    mod = types.ModuleType("antenv.axon_hooks")
    state = {"hook": hook}
    mod.get_axon_ntff_profile_hook = lambda: state["hook"]
    mod.set_axon_ntff_profile_hook = lambda h: state.__setitem__("hook", h)
    import antenv

    sys.modules["antenv.axon_hooks"] = mod
    antenv.axon_hooks = mod


def _build():
    nc = bacc.Bacc("TRN2", target_bir_lowering=False, debug=False, num_devices=8)
    xt_d = nc.dram_tensor("xt", [C, N], BF16, kind="ExternalInput").ap()
    w_d = nc.dram_tensor("w", [C, C], BF16, kind="ExternalInput").ap()
    out_d = nc.dram_tensor("out", [SLAB, N], F32, kind="ExternalOutput").ap()

    NJ = N // NT  # column chunks

    with tile.TileContext(nc) as tc:
        with (
            tc.tile_pool(name="singles", bufs=1) as singles,
            tc.tile_pool(name="ppq", bufs=3, space="PSUM") as ppq,
            tc.tile_pool(name="ppz", bufs=2, space="PSUM") as ppz,
            tc.tile_pool(name="pps", bufs=1, space="PSUM") as pps,
            tc.tile_pool(name="outs", bufs=4) as outs,
        ):
            w_sb = singles.tile([C, C], BF16)
            nc.sync.dma_start(w_sb[:], w_d[:, :])
            xt_sb = singles.tile([C, N], BF16)
            for jc in range(NJ):
                nc.sync.dma_start(
                    xt_sb[:, bass.ts(jc, NT)], xt_d[:, bass.ts(jc, NT)]
                )

            # augmented Z^T: rows 0..63 = Z^T (bf16), row 64 = -d row
            zt = singles.tile([C + 1, N], BF16)
            # augmented lhsT: rows 0..63 = 2*Z^T[:, :SLAB], row 64 = ones
            zt2 = singles.tile([C + 1, SLAB], BF16)
            nc.vector.memset(zt2[C : C + 1, :], 1.0)
            ones_c = singles.tile([C, 1], BF16)
            nc.vector.memset(ones_c[:], 1.0)

            sq = singles.tile([C, N], BF16)
            dsq = singles.tile([P, 2], F32)
            ndi = singles.tile([P, 2], F32)
            sqr_scratch = singles.tile([P, C], F32)

            for jc in range(NJ):
                pz = ppz.tile([C, NT], F32, tag="pz")
                nc.tensor.matmul(
                    pz[:], w_sb[:], xt_sb[:, bass.ts(jc, NT)], start=True, stop=True
                )
                nc.vector.tensor_copy(zt[0:C, bass.ts(jc, NT)], pz[:])
                if jc == 0:
                    # 2*Z^T for the slab's rows (first SLAB rotated columns)
                    nc.scalar.mul(zt2[0:C, :], pz[:, 0:SLAB], 2.0)
                # squared entries -> column sums d
                nc.vector.tensor_mul(
                    sq[:, bass.ts(jc, NT)],
                    zt[0:C, bass.ts(jc, NT)],
                    zt[0:C, bass.ts(jc, NT)],
                )
                pd = pps.tile([1, NT], F32, tag="pd")
                nc.tensor.matmul(
                    pd[:], ones_c[:], sq[:, bass.ts(jc, NT)], start=True, stop=True
                )
                nc.scalar.mul(zt[C : C + 1, bass.ts(jc, NT)], pd[:], -1.0)

            # d_i for the slab rows, in row layout: Zrow = X_slab @ W
            for g in range(2):
                pzr = pps.tile([P, C], F32, tag="pzr")
                nc.tensor.matmul(
                    pzr[:], xt_sb[:, bass.ts(g, P)], w_sb[:], start=True, stop=True
                )
                nc.scalar.activation(
                    sqr_scratch[:],
                    pzr[:],
                    mybir.ActivationFunctionType.Square,
                    accum_out=dsq[:, g : g + 1],
                )
            nc.vector.tensor_scalar_mul(ndi[:], dsq[:], -1.0)

            for g in range(2):
                for jc in range(NJ):
                    pq = ppq.tile([P, NT], F32, tag="pq")
                    # pq = 2 G - d_j   (row 64 of zt/zt2 carries the -d row)
                    nc.tensor.matmul(
                        pq[:],
                        zt2[:, bass.ts(g, P)],
                        zt[:, bass.ts(jc, NT)],
                        start=True,
                        stop=True,
                    )
                    ot = outs.tile([P, NT], F32, tag="ot")
                    # ot = exp(pq - d_i)
                    nc.scalar.activation(
                        ot[:],
                        pq[:],
                        mybir.ActivationFunctionType.Exp,
                        bias=ndi[:, g : g + 1],
                        scale=1.0,
                    )
                    if jc == 0:
                        # rotated diagonal block at local col == local row:
                        # exact exp(0) + 1 = 2.0
                        nc.gpsimd.affine_select(
                            out=ot[:, bass.ts(g, P)],
                            in_=ot[:, bass.ts(g, P)],
                            compare_op=mybir.AluOpType.not_equal,
                            fill=2.0,
                            base=0,
                            pattern=[[-1, P]],
                            channel_multiplier=1,
                        )
                    nc.sync.dma_start(out_d[bass.ts(g, P), bass.ts(jc, NT)], ot[:])

    nc.compile()
    return nc


def _get_nc():
    global _NC
    if _NC is None:
        _NC = _build()
    return _NC


def kernel(regional_means, W, c=None, **_kw):
    global LAST_EXEC_NS
    x = np.ascontiguousarray(np.asarray(regional_means, dtype=np.float32))
    w = np.ascontiguousarray(np.asarray(W, dtype=np.float32))
    assert x.shape == (B, N, C) and w.shape == (C, C)

    nc = _get_nc()
    w_bf = w.astype(ml_dtypes.bfloat16)
    in_maps = []
    for k in range(8):
        b, s = divmod(k, 4)
        row0 = s * SLAB
        xt_rot = np.roll(x[b].T, -row0, axis=1)
        in_maps.append(
            {"xt": np.ascontiguousarray(xt_rot.astype(ml_dtypes.bfloat16)), "w": w_bf}
        )

    if TRACE:
        _ensure_ntff_hook()
    res = run_bass_kernel_spmd(nc, in_maps, core_ids=list(range(8)), trace=TRACE)
    LAST_EXEC_NS = res.exec_time_ns

    adj = np.empty((B, N, N), dtype=np.float32)
    for k in range(8):
        b, s = divmod(k, 4)
        row0 = s * SLAB
        adj[b, row0 : row0 + SLAB, :] = np.roll(res.results[k]["out"], row0, axis=1)
    return adj
